# revision 39
# baseline (speedup 1.0000x reference)
"""AmbientReflectionNet Trainium2 kernel (8 NeuronCores, data parallel).

Reference computation (per point):
  n = l2norm(normals); v = l2norm(view_dirs)
  visible = dot(n, v) > 0
  diffuse  = visible ? MLP_d(n)              : 0   (3->256->256->256->3, ReLU)
  specular = visible ? MLP_s([n,v,rough,r0]) : 0   (8->256->256->256->3, ReLU)

The original module is gather->MLP->scatter: only visible points (~50%)
contribute output. We exploit that at the sharding layer: the host routes
only points with dot(normals, view_dirs) > -eps to the device (compacted,
padded to a whole number of 512-point tiles, split across 8 cores), the
device runs the full normalize+mask+MLP pipeline on what it receives, and
the host scatters results into a zero output. Invisible points are exactly
zero in the reference, so outputs are unchanged.

Device layout per 512-point tile:
  - load point-major [128, 8, 8] tiles, normalize + mask on DVE/ACT
  - A columns: n(3), v(3), ro, r0, mask(x3) -> PE-transpose to [11, 512]
    (the 3 replicated mask columns land on partitions 8:11, giving the
    3-partition mask operand for the final epilogue with no shuffle)
  - MLP layers as feature-major fp16 matmuls (L0: d rows 0-2 / s rows 64-71
    run as concurrent PE row-tiles)
  - bias+ReLU epilogues split across ScalarE (ACT, diffuse) and VectorE
    (DVE, specular); GpSimd (SBUF-only) runs the normalize prep and the
    final mask multiply so no single engine gates the PE
  - layer 3 for both MLPs lands in one shared psum tile (PE column tiles
    0-3 / 32-35), merged mask epilogues over the tile pair, one output
    DMA per network per tile pair, feature-major
"""

import numpy as np

import concourse.bass as bass
import concourse.mybir as mybir
import concourse.tile as tile
from concourse import bacc
from concourse.bass_utils import run_bass_kernel_spmd

NCORES = 8
P_FULL = 262144
TILE = 512
DEFAULT_NT = 32  # tiles per core (compacted); must be even
H = 256
F32 = mybir.dt.float32
FP16 = mybir.dt.float16
EPS = 1e-12
DOT_MARGIN = 1e-5  # host routes dot > -margin; device mask decides exactly

_CACHE = {}


def _build(nt):
    from contextlib import ExitStack

    assert nt % 2 == 0
    ppc = nt * TILE

    nc = bacc.Bacc()

    pts = nc.declare_dram_parameter("pts", [ppc, 8], F32, isOutput=False)
    identb_in = nc.declare_dram_parameter("identb", [128, 128], FP16, isOutput=False)

    w0pack_in = nc.declare_dram_parameter("W0pack", [128, 2, 128], FP16, isOutput=False)
    dWp = {
        ("d", 1): nc.declare_dram_parameter("dW1p", [H, H], FP16, isOutput=False),
        ("s", 1): nc.declare_dram_parameter("sW1p", [H, H], FP16, isOutput=False),
        ("d", 2): nc.declare_dram_parameter("dW2p", [H, H], FP16, isOutput=False),
        ("s", 2): nc.declare_dram_parameter("sW2p", [H, H], FP16, isOutput=False),
        ("d", 3): nc.declare_dram_parameter("dW3p", [H, 4], FP16, isOutput=False),
        ("s", 3): nc.declare_dram_parameter("sW3p", [H, 4], FP16, isOutput=False),
    }
    dB = {}
    for pfx in ("d", "s"):
        for i in range(4):
            n = H if i < 3 else 3
            dB[pfx, i] = nc.declare_dram_parameter(
                f"{pfx}b{i}", [n], F32, isOutput=False
            )

    out_d = nc.declare_dram_parameter("out_d", [3, ppc], F32, isOutput=True)
    out_s = nc.declare_dram_parameter("out_s", [3, ppc], F32, isOutput=True)

    with tile.TileContext(nc) as tc, ExitStack() as ctx:
        const = ctx.enter_context(tc.tile_pool(name="const", bufs=1))
        pool_in = ctx.enter_context(tc.tile_pool(name="pin", bufs=3))
        pool_araw = ctx.enter_context(tc.tile_pool(name="paraw", bufs=1))
        pool_rhs = ctx.enter_context(tc.tile_pool(name="prhs", bufs=3))
        pool_h = ctx.enter_context(tc.tile_pool(name="ph", bufs=2))
        pool_out = ctx.enter_context(tc.tile_pool(name="pout", bufs=3))
        ps_tr = ctx.enter_context(tc.tile_pool(name="pstr", bufs=1, space="PSUM"))
        ps_mm = {
            "d": ctx.enter_context(tc.tile_pool(name="psmmd", bufs=2, space="PSUM")),
            "s": ctx.enter_context(tc.tile_pool(name="psmms", bufs=3, space="PSUM")),
        }
        ps_l3 = ctx.enter_context(tc.tile_pool(name="psl3", bufs=1, space="PSUM"))

        # ---- constants ----
        identb = const.tile([128, 128], FP16)
        nc.sync.dma_start(identb, identb_in[:, :])

        # layer-0 weights, row-packed: rows 0-2 diffuse (n), rows 64-71
        # specular (n+v+ro+r0); [k, half, m]
        W0pack = const.tile([128, 2, 128], FP16, name="W0pack")
        nc.sync.dma_start(W0pack, w0pack_in[:, :, :])

        # mid layer weights [128, chunk, 256]
        Wmid = {}
        for pfx in ("d", "s"):
            for li in (1, 2):
                w = const.tile([128, 2, H], FP16, name=f"W{li}{pfx}")
                nc.sync.dma_start(w, dWp[pfx, li].rearrange("(c p) m -> p c m", p=128))
                Wmid[pfx, li] = w

        # last layer weights [128, chunk, 4] (output dim padded to 4)
        W3 = {}
        for pfx in ("d", "s"):
            w = const.tile([128, 2, 4], FP16, name=f"W3{pfx}")
            nc.sync.dma_start(w, dWp[pfx, 3].rearrange("(c p) m -> p c m", p=128))
            W3[pfx] = w

        # biases for layers 0..2: [128, half]; layer 3: [3, 1]
        Bias = {}
        for pfx in ("d", "s"):
            for li in (0, 1, 2):
                b = const.tile([128, 2], F32, name=f"B{li}{pfx}")
                nc.sync.dma_start(b, dB[pfx, li].rearrange("(h p) -> p h", p=128))
                Bias[pfx, li] = b
            b = const.tile([3, 1], F32, name=f"B3{pfx}")
            nc.sync.dma_start(b, dB[pfx, 3].rearrange("(c o) -> c o", o=1))
            Bias[pfx, 3] = b

        # pre-warm PE's view of the constant DMAs so steady-state matmuls
        # and transposes never carry a DMA-queue wait
        wtile = ps_mm["d"].tile([128, 512], F32, tag="mm", name="wtile")
        warm = wtile[:, 0:128]
        nc.tensor.matmul(warm, identb, identb, start=True, stop=True)
        nc.tensor.matmul(warm, W0pack[:, 0, :], identb, start=True, stop=True)
        for wt in (
            Wmid["d", 1][:, 0, 0:128],
            Wmid["s", 1][:, 0, 0:128],
            Wmid["d", 2][:, 0, 0:128],
            Wmid["s", 2][:, 0, 0:128],
            W3["d"][:, 0, :],
            W3["s"][:, 0, :],
        ):
            kp, fp = wt.shape
            nc.tensor.matmul(
                warm[0:fp, :], wt, identb[0:kp, :], start=True, stop=True
            )

        # epilogue engine assignment: PSUM is only reachable from ACT/DVE.
        # Cross the mapping by u — (d,u0)/(s,u1) on ACT, (d,u1)/(s,u0) on
        # DVE — so each network's two per-u epilogues run on different
        # engines in parallel and the full h pair lands ~1us earlier.
        # GpSimd (SBUF-only) takes the normalize prep + final mask multiply.
        def relu_epilogue(dst, psrc, bias_ap, key):
            pfx, li, half, u = key
            if (pfx == "d") == (u == 0):
                nc.scalar.activation(
                    dst, psrc, mybir.ActivationFunctionType.Relu, bias=bias_ap
                )
            else:
                nc.vector.tensor_scalar(
                    dst, psrc, bias_ap, 0.0, mybir.AluOpType.add, mybir.AluOpType.max
                )

        pts_pm2 = pts.rearrange("(t g p) c -> t p g c", p=128, g=8)
        for tp in range(nt // 2):
            # ---- load two tiles point-major [128, 8, 8]; prep batched ----
            Araw = pool_araw.tile(
                [128, 8, 8], F32, tag=f"araw{tp}", name=f"araw{tp}"
            )
            nc.sync.dma_start(Araw, pts_pm2[tp])

            S = pool_in.tile([128, 8, 9], F32, name="S")
            nc.gpsimd.tensor_tensor(
                S[:, :, 0:6], Araw[:, :, 0:6], Araw[:, :, 0:6], mybir.AluOpType.mult
            )
            nc.gpsimd.tensor_tensor(
                S[:, :, 6:9], Araw[:, :, 0:3], Araw[:, :, 3:6], mybir.AluOpType.mult
            )
            R = pool_in.tile([128, 8, 3], F32, name="R")
            nc.vector.tensor_reduce(
                R,
                S.rearrange("p g (q c) -> p g q c", c=3),
                axis=mybir.AxisListType.X,
                op=mybir.AluOpType.add,
            )
            # A cols: n(3), v(3), ro, r0, mask(x3)
            A = pool_in.tile([128, 8, 11], FP16, name="A")
            nc.gpsimd.tensor_scalar(
                A[:, :, 8:11],
                R[:, :, 2:3].to_broadcast([128, 8, 3]),
                0.0,
                None,
                mybir.AluOpType.is_gt,
            )
            nc.scalar.activation(
                R[:, :, 0:2], R[:, :, 0:2], mybir.ActivationFunctionType.Sqrt
            )
            nc.vector.tensor_scalar_max(R[:, :, 0:2], R[:, :, 0:2], EPS)
            nc.vector.reciprocal(R[:, :, 0:2], R[:, :, 0:2])
            nc.gpsimd.tensor_tensor(
                A[:, :, 0:3],
                Araw[:, :, 0:3],
                R[:, :, 0:1].to_broadcast([128, 8, 3]),
                mybir.AluOpType.mult,
            )
            nc.gpsimd.tensor_tensor(
                A[:, :, 3:6],
                Araw[:, :, 3:6],
                R[:, :, 1:2].to_broadcast([128, 8, 3]),
                mybir.AluOpType.mult,
            )
            nc.gpsimd.tensor_scalar_mul(A[:, :, 6:8], Araw[:, :, 6:8], 1.0)

            # ---- transposes for both tiles of the pair into one psum bank ----
            ptr = ps_tr.tile([11, 2, 512], FP16, tag="tr", name="ptr")
            for u in range(2):
                for g in range(4):
                    nc.tensor.transpose(
                        ptr[:, u, g * 128 : (g + 1) * 128],
                        A[:, 4 * u + g, 0:11],
                        identb,
                    )
            # rhs0 rows: 0:3 n, 3:6 v, 6 ro, 7 r0, 8:11 mask;
            # rows 64:72 = specular inputs (n, v, ro, r0)
            rhs0 = pool_rhs.tile([72, 2, 512], FP16, tag="rhs0")
            # both copies read the transpose psum directly and run on
            # different engines in parallel
            nc.vector.tensor_copy(rhs0[0:11, :, :], ptr)
            nc.scalar.activation(
                rhs0[64:72, :, :],
                ptr[0:8, :, :],
                mybir.ActivationFunctionType.Copy,
            )
            # partition-0-aligned mask copy for the final epilogues
            mb2 = pool_rhs.tile([3, 2, 512], FP16, tag="mb2")
            nc.sync.dma_start(mb2, rhs0[8:11, :, :])

            def new_h(pfx, li):
                return pool_h.tile(
                    [128, 2, 2, 512], FP16, tag=f"h{li}{pfx}", name=f"h{li}{pfx}"
                )

            # ---- layer 0: diffuse (rows 0-2) and specular (rows 64-71)
            # run as concurrent row-tiles of the PE array; per-u psum tiles
            # keep epilogue latency low ----
            hcur = {pfx: new_h(pfx, 1) for pfx in ("d", "s")}
            for half in range(2):
                for u in range(2):
                    ps0 = ps_mm["d"].tile([128, 512], F32, tag="mm", name="ps0")
                    pss = ps_mm["s"].tile([128, 512], F32, tag="mm", name="pss")
                    nc.tensor.matmul(
                        ps0, W0pack[0:3, half, :], rhs0[0:3, u, :],
                        start=True, stop=True, tile_position=(0, 0),
                    )
                    nc.tensor.matmul(
                        pss, W0pack[64:72, half, :], rhs0[64:72, u, :],
                        start=True, stop=True, tile_position=(64, 0),
                    )
                    relu_epilogue(
                        hcur["d"][:, half, u, :], ps0,
                        Bias["d", 0][:, half : half + 1], ("d", 0, half, u),
                    )
                    relu_epilogue(
                        hcur["s"][:, half, u, :], pss,
                        Bias["s", 0][:, half : half + 1], ("s", 0, half, u),
                    )

            # ---- layers 1, 2: same weights serve both tiles back-to-back,
            # per-u psums + epilogues ----
            for li in (1, 2):
                hnext = {pfx: new_h(pfx, li + 1) for pfx in ("d", "s")}
                for half in range(2):
                    for pfx in ("d", "s"):
                        psu = [
                            ps_mm[pfx].tile([128, 512], F32, tag="mm", name="ps")
                            for _ in range(2)
                        ]
                        for c in range(2):
                            for u in range(2):
                                nc.tensor.matmul(
                                    psu[u],
                                    Wmid[pfx, li][:, c, half * 128 : half * 128 + 128],
                                    hcur[pfx][:, c, u, :],
                                    start=(c == 0),
                                    stop=(c == 1),
                                )
                        for u in range(2):
                            relu_epilogue(
                                hnext[pfx][:, half, u, :],
                                psu[u],
                                Bias[pfx, li][:, half : half + 1],
                                (pfx, li, half, u),
                            )
                hcur = hnext

            # ---- layer 3 (d at PE columns 0-3, s at columns 32-35, both
            # into one shared psum tile) + merged mask epilogues ----
            ps3 = ps_l3.tile([36, 2, 512], F32, tag="l3", name="ps3")
            for u in range(2):
                for c in range(2):
                    nc.tensor.matmul(
                        ps3[0:4, u, :],
                        W3["d"][:, c, :],
                        hcur["d"][:, c, u, :],
                        start=(c == 0), stop=(c == 1), tile_position=(0, 0),
                    )
                for c in range(2):
                    nc.tensor.matmul(
                        ps3[32:36, u, :],
                        W3["s"][:, c, :],
                        hcur["s"][:, c, u, :],
                        start=(c == 0), stop=(c == 1), tile_position=(0, 32),
                    )
            # d: ACT adds bias psum->sbuf, GpSimd applies the mask;
            # s: DVE does (psum + b) * mask in one op
            ot = pool_out.tile([3, 2, 512], F32, tag="otmp")
            nc.scalar.activation(
                ot,
                ps3[0:3, :, :],
                mybir.ActivationFunctionType.Identity,
                bias=Bias["d", 3][:, 0:1],
            )
            osb_d = pool_out.tile([3, 2, 512], F32, tag="od")
            nc.gpsimd.tensor_tensor(osb_d, ot, mb2, mybir.AluOpType.mult)
            osb_s = pool_out.tile([3, 2, 512], F32, tag="os")
            nc.vector.scalar_tensor_tensor(
                osb_s,
                ps3[32:35, :, :],
                Bias["s", 3][:, 0:1],
                mb2,
                mybir.AluOpType.add,
                mybir.AluOpType.mult,
            )
            for pfx, osb, outbuf in (("d", osb_d, out_d), ("s", osb_s, out_s)):
                nc.sync.dma_start(
                    outbuf[:, tp * 2 * TILE : (tp + 1) * 2 * TILE].rearrange(
                        "p (a b) -> p a b", b=TILE
                    ),
                    osb,
                )

    nc.compile()
    return nc


def _pack_weights(inputs):
    """Pad + fp16-cast the weight matrices once (shared across cores)."""
    w = {}
    d0 = np.asarray(inputs["dW0"], np.float32)  # [3, H]
    s0 = np.asarray(inputs["sW0"], np.float32)  # [8, H]
    pack = np.zeros((128, 2, 128), np.float32)
    for h in range(2):
        pack[0:3, h, :] = d0[:, h * 128 : h * 128 + 128]
        pack[64:72, h, :] = s0[:, h * 128 : h * 128 + 128]
    w["W0pack"] = pack.astype(np.float16)

    bf = np.float16
    for pfx in ("d", "s"):
        for li in (1, 2):
            w[f"{pfx}W{li}p"] = np.asarray(inputs[f"{pfx}W{li}"], dtype=bf)
        w[f"{pfx}W3p"] = np.asarray(
            np.concatenate(
                [inputs[f"{pfx}W3"], np.zeros((H, 1), np.float32)], axis=1
            ),
            dtype=bf,
        )  # [H, 4]
        for li in range(4):
            w[f"{pfx}b{li}"] = np.ascontiguousarray(
                inputs[f"{pfx}b{li}"], dtype=np.float32
            )
    return w


def get_nc(nt=DEFAULT_NT):
    key = ("nc", nt)
    if key not in _CACHE:
        _CACHE[key] = _build(nt)
    return _CACHE[key]


def _required_nt(nv):
    """Tiles per core needed for nv compacted points (rounded up to even)."""
    nt = -(-nv // (NCORES * TILE))
    nt += nt % 2
    return max(nt, 2)


def make_shards(inputs, nt=DEFAULT_NT):
    """Compact visible points, pad to nt tiles/core, build per-core shards.

    vis_idx is stashed in _CACHE for gather_outputs so the test harness's
    shard->run->gather flow works.
    """
    wpack = _pack_weights(inputs)
    pts_all = np.ascontiguousarray(
        np.concatenate(
            [
                np.asarray(inputs["normals"], np.float32),
                np.asarray(inputs["view_dirs"], np.float32),
                np.asarray(inputs["roughness"], np.float32),
                np.asarray(inputs["r0"], np.float32),
            ],
            axis=1,
        )
    )
    dot = np.einsum("ij,ij->i", pts_all[:, 0:3], pts_all[:, 3:6], dtype=np.float32)
    vis_idx = np.nonzero(dot > -DOT_MARGIN)[0]
    nv = len(vis_idx)
    need = _required_nt(nv)
    assert need <= nt, (
        f"visible points {nv} need {need} tiles/core but kernel built for {nt}"
    )
    ppc = nt * TILE
    pts_vis = np.zeros((NCORES * ppc, 8), np.float32)
    pts_vis[:nv] = pts_all[vis_idx]

    ident_bf = np.eye(128, dtype=np.float16)
    shards = []
    for i in range(NCORES):
        m = {"pts": pts_vis[i * ppc : (i + 1) * ppc], "identb": ident_bf}
        m.update(wpack)
        shards.append(m)
    _CACHE["vis_idx"] = vis_idx
    _CACHE["ppc"] = ppc
    return shards


def gather_outputs(results):
    vis_idx = _CACHE["vis_idx"]
    ppc = _CACHE["ppc"]
    nv = len(vis_idx)
    diff = np.zeros((P_FULL, 3), np.float32)
    spec = np.zeros((P_FULL, 3), np.float32)
    for i in range(NCORES):
        lo = i * ppc
        hi = min(lo + ppc, nv)
        if hi <= lo:
            break
        sl = vis_idx[lo:hi]
        diff[sl] = results[i]["out_d"][:, : hi - lo].T
        spec[sl] = results[i]["out_s"][:, : hi - lo].T
    return diff, spec


def kernel(**inputs):
    dot = np.einsum(
        "ij,ij->i",
        np.asarray(inputs["normals"], np.float32),
        np.asarray(inputs["view_dirs"], np.float32),
    )
    nv = int((dot > -DOT_MARGIN).sum())
    nt = max(_required_nt(nv), DEFAULT_NT)
    nc = get_nc(nt)
    shards = make_shards(inputs, nt)
    res = run_bass_kernel_spmd(nc, shards, core_ids=list(range(NCORES)))
    return gather_outputs(res.results)


# revision 41
# speedup vs baseline: 1.1643x; 1.1643x over previous
"""AmbientReflectionNet Trainium2 kernel (8 NeuronCores, data parallel).

Reference computation (per point):
  n = l2norm(normals); v = l2norm(view_dirs)
  visible = dot(n, v) > 0
  diffuse  = visible ? MLP_d(n)              : 0   (3->256->256->256->3, ReLU)
  specular = visible ? MLP_s([n,v,rough,r0]) : 0   (8->256->256->256->3, ReLU)

The original module is gather->MLP->scatter: only visible points (~50%)
contribute output. We exploit that at the sharding layer: the host routes
only points with dot(normals, view_dirs) > -eps to the device (compacted,
padded to a whole number of 512-point tiles, split across 8 cores), the
device runs the full normalize+mask+MLP pipeline on what it receives, and
the host scatters results into a zero output. Invisible points are exactly
zero in the reference, so outputs are unchanged.

Device layout per 512-point tile:
  - load point-major [128, 8, 8] tiles, normalize + mask on DVE/ACT
  - A columns: n(3), v(3), ro, r0, mask(x3) -> PE-transpose to [11, 512]
    (the 3 replicated mask columns land on partitions 8:11, giving the
    3-partition mask operand for the final epilogue with no shuffle)
  - MLP layers as feature-major fp16 matmuls (L0: d rows 0-2 / s rows 64-71
    run as concurrent PE row-tiles)
  - bias+ReLU epilogues split across ScalarE (ACT, diffuse) and VectorE
    (DVE, specular); GpSimd (SBUF-only) runs the normalize prep and the
    final mask multiply so no single engine gates the PE
  - layer 3 for both MLPs lands in one shared psum tile (PE column tiles
    0-3 / 32-35), merged mask epilogues over the tile pair, one output
    DMA per network per tile pair, feature-major
"""

import numpy as np

import concourse.bass as bass
import concourse.mybir as mybir
import concourse.tile as tile
from concourse import bacc
from concourse.bass_utils import run_bass_kernel_spmd

NCORES = 8
P_FULL = 262144
TILE = 512
DEFAULT_NT = 32  # tiles per core (compacted); must be even
H = 256
F32 = mybir.dt.float32
FP16 = mybir.dt.float16
EPS = 1e-12
DOT_MARGIN = 1e-5  # host routes dot > -margin; device mask decides exactly

_CACHE = {}


def _build(nt):
    from contextlib import ExitStack

    assert nt % 2 == 0
    ppc = nt * TILE

    nc = bacc.Bacc()

    pts = nc.declare_dram_parameter("pts", [ppc, 8], F32, isOutput=False)
    identb_in = nc.declare_dram_parameter("identb", [128, 128], FP16, isOutput=False)

    w0pack_in = nc.declare_dram_parameter("W0pack", [128, 2, 128], FP16, isOutput=False)
    dWp = {
        ("d", 1): nc.declare_dram_parameter("dW1p", [H, H], FP16, isOutput=False),
        ("s", 1): nc.declare_dram_parameter("sW1p", [H, H], FP16, isOutput=False),
        ("d", 2): nc.declare_dram_parameter("dW2p", [H, H], FP16, isOutput=False),
        ("s", 2): nc.declare_dram_parameter("sW2p", [H, H], FP16, isOutput=False),
        ("d", 3): nc.declare_dram_parameter("dW3p", [H, 4], FP16, isOutput=False),
        ("s", 3): nc.declare_dram_parameter("sW3p", [H, 4], FP16, isOutput=False),
    }
    dB = {}
    for pfx in ("d", "s"):
        for i in range(4):
            n = H if i < 3 else 3
            dB[pfx, i] = nc.declare_dram_parameter(
                f"{pfx}b{i}", [n], F32, isOutput=False
            )

    out_d = nc.declare_dram_parameter("out_d", [3, ppc], F32, isOutput=True)
    out_s = nc.declare_dram_parameter("out_s", [3, ppc], F32, isOutput=True)

    with tile.TileContext(nc) as tc, ExitStack() as ctx:
        const = ctx.enter_context(tc.tile_pool(name="const", bufs=1))
        pool_in = ctx.enter_context(tc.tile_pool(name="pin", bufs=3))
        pool_araw = ctx.enter_context(tc.tile_pool(name="paraw", bufs=1))
        pool_rhs = ctx.enter_context(tc.tile_pool(name="prhs", bufs=3))
        pool_h = ctx.enter_context(tc.tile_pool(name="ph", bufs=2))
        pool_out = ctx.enter_context(tc.tile_pool(name="pout", bufs=3))
        ps_tr = ctx.enter_context(tc.tile_pool(name="pstr", bufs=1, space="PSUM"))
        ps_mm = {
            "d": ctx.enter_context(tc.tile_pool(name="psmmd", bufs=2, space="PSUM")),
            "s": ctx.enter_context(tc.tile_pool(name="psmms", bufs=3, space="PSUM")),
        }
        ps_l3 = ctx.enter_context(tc.tile_pool(name="psl3", bufs=1, space="PSUM"))

        # ---- constants ----
        identb = const.tile([128, 128], FP16)
        nc.sync.dma_start(identb, identb_in[:, :])

        # layer-0 weights, row-packed: rows 0-2 diffuse (n), rows 64-71
        # specular (n+v+ro+r0); [k, half, m]
        W0pack = const.tile([128, 2, 128], FP16, name="W0pack")
        nc.sync.dma_start(W0pack, w0pack_in[:, :, :])

        # mid layer weights [128, chunk, 256]
        Wmid = {}
        for pfx in ("d", "s"):
            for li in (1, 2):
                w = const.tile([128, 2, H], FP16, name=f"W{li}{pfx}")
                nc.sync.dma_start(w, dWp[pfx, li].rearrange("(c p) m -> p c m", p=128))
                Wmid[pfx, li] = w

        # last layer weights [128, chunk, 4] (output dim padded to 4)
        W3 = {}
        for pfx in ("d", "s"):
            w = const.tile([128, 2, 4], FP16, name=f"W3{pfx}")
            nc.sync.dma_start(w, dWp[pfx, 3].rearrange("(c p) m -> p c m", p=128))
            W3[pfx] = w

        # biases for layers 0..2: [128, half]; layer 3: [3, 1]
        Bias = {}
        for pfx in ("d", "s"):
            for li in (0, 1, 2):
                b = const.tile([128, 2], F32, name=f"B{li}{pfx}")
                nc.sync.dma_start(b, dB[pfx, li].rearrange("(h p) -> p h", p=128))
                Bias[pfx, li] = b
            b = const.tile([3, 1], F32, name=f"B3{pfx}")
            nc.sync.dma_start(b, dB[pfx, 3].rearrange("(c o) -> c o", o=1))
            Bias[pfx, 3] = b

        # pre-warm PE's view of the constant DMAs so steady-state matmuls
        # and transposes never carry a DMA-queue wait
        wtile = ps_mm["d"].tile([128, 512], F32, tag="mm", name="wtile")
        warm = wtile[:, 0:128]
        nc.tensor.matmul(warm, identb, identb, start=True, stop=True)
        nc.tensor.matmul(warm, W0pack[:, 0, :], identb, start=True, stop=True)
        for wt in (
            Wmid["d", 1][:, 0, 0:128],
            Wmid["s", 1][:, 0, 0:128],
            Wmid["d", 2][:, 0, 0:128],
            Wmid["s", 2][:, 0, 0:128],
            W3["d"][:, 0, :],
            W3["s"][:, 0, :],
        ):
            kp, fp = wt.shape
            nc.tensor.matmul(
                warm[0:fp, :], wt, identb[0:kp, :], start=True, stop=True
            )

        # epilogue engine assignment: PSUM is only reachable from ACT/DVE.
        # Cross the mapping by u — (d,u0)/(s,u1) on ACT, (d,u1)/(s,u0) on
        # DVE — so each network's two per-u epilogues run on different
        # engines in parallel and the full h pair lands ~1us earlier.
        # GpSimd (SBUF-only) takes the normalize prep + final mask multiply.
        def relu_epilogue(dst, psrc, bias_ap, key):
            pfx, li, half, u = key
            if (pfx == "d") == (u == 0):
                nc.scalar.activation(
                    dst, psrc, mybir.ActivationFunctionType.Relu, bias=bias_ap
                )
            else:
                nc.vector.tensor_scalar(
                    dst, psrc, bias_ap, 0.0, mybir.AluOpType.add, mybir.AluOpType.max
                )

        pts_pm2 = pts.rearrange("(t g p) c -> t p g c", p=128, g=8)
        # fire all input DMAs upfront: the tiles are statically allocated
        # (unique tags) and this keeps the Pool queue from pacing them —
        # data lands well before each tile-pair's prep chain needs it
        Araws = []
        for tp in range(nt // 2):
            Araw = pool_araw.tile(
                [128, 8, 8], F32, tag=f"araw{tp}", name=f"araw{tp}"
            )
            nc.gpsimd.dma_start(Araw, pts_pm2[tp])
            Araws.append(Araw)
        for tp in range(nt // 2):
            # ---- two tiles point-major [128, 8, 8]; prep batched ----
            Araw = Araws[tp]

            S = pool_in.tile([128, 8, 9], F32, name="S")
            nc.gpsimd.tensor_tensor(
                S[:, :, 0:6], Araw[:, :, 0:6], Araw[:, :, 0:6], mybir.AluOpType.mult
            )
            nc.gpsimd.tensor_tensor(
                S[:, :, 6:9], Araw[:, :, 0:3], Araw[:, :, 3:6], mybir.AluOpType.mult
            )
            R = pool_in.tile([128, 8, 3], F32, name="R")
            nc.vector.tensor_reduce(
                R,
                S.rearrange("p g (q c) -> p g q c", c=3),
                axis=mybir.AxisListType.X,
                op=mybir.AluOpType.add,
            )
            # A cols: n(3), v(3), ro, r0, mask(x3)
            A = pool_in.tile([128, 8, 11], FP16, name="A")
            nc.gpsimd.tensor_scalar(
                A[:, :, 8:11],
                R[:, :, 2:3].to_broadcast([128, 8, 3]),
                0.0,
                None,
                mybir.AluOpType.is_gt,
            )
            nc.scalar.activation(
                R[:, :, 0:2], R[:, :, 0:2], mybir.ActivationFunctionType.Sqrt
            )
            nc.vector.tensor_scalar_max(R[:, :, 0:2], R[:, :, 0:2], EPS)
            nc.vector.reciprocal(R[:, :, 0:2], R[:, :, 0:2])
            nc.gpsimd.tensor_tensor(
                A[:, :, 0:3],
                Araw[:, :, 0:3],
                R[:, :, 0:1].to_broadcast([128, 8, 3]),
                mybir.AluOpType.mult,
            )
            nc.gpsimd.tensor_tensor(
                A[:, :, 3:6],
                Araw[:, :, 3:6],
                R[:, :, 1:2].to_broadcast([128, 8, 3]),
                mybir.AluOpType.mult,
            )
            nc.gpsimd.tensor_scalar_mul(A[:, :, 6:8], Araw[:, :, 6:8], 1.0)

            # ---- transposes for both tiles of the pair into one psum bank ----
            ptr = ps_tr.tile([11, 2, 512], FP16, tag="tr", name="ptr")
            for u in range(2):
                for g in range(4):
                    nc.tensor.transpose(
                        ptr[:, u, g * 128 : (g + 1) * 128],
                        A[:, 4 * u + g, 0:11],
                        identb,
                    )
            # rhs0 rows: 0:3 n, 3:6 v, 6 ro, 7 r0, 8:11 mask;
            # rows 64:72 = specular inputs (n, v, ro, r0)
            rhs0 = pool_rhs.tile([72, 2, 512], FP16, tag="rhs0")
            # both copies read the transpose psum directly and run on
            # different engines in parallel
            nc.vector.tensor_copy(rhs0[0:11, :, :], ptr)
            nc.scalar.activation(
                rhs0[64:72, :, :],
                ptr[0:8, :, :],
                mybir.ActivationFunctionType.Copy,
            )
            # partition-0-aligned mask copy for the final epilogues
            mb2 = pool_rhs.tile([3, 2, 512], FP16, tag="mb2")
            nc.sync.dma_start(mb2, rhs0[8:11, :, :])

            def new_h(pfx, li):
                return pool_h.tile(
                    [128, 2, 2, 512], FP16, tag=f"h{li}{pfx}", name=f"h{li}{pfx}"
                )

            # ---- layer 0: diffuse (rows 0-2) and specular (rows 64-71)
            # run as concurrent row-tiles of the PE array; per-u psum tiles
            # keep epilogue latency low ----
            hcur = {pfx: new_h(pfx, 1) for pfx in ("d", "s")}
            for half in range(2):
                for u in range(2):
                    ps0 = ps_mm["d"].tile([128, 512], F32, tag="mm", name="ps0")
                    pss = ps_mm["s"].tile([128, 512], F32, tag="mm", name="pss")
                    nc.tensor.matmul(
                        ps0, W0pack[0:3, half, :], rhs0[0:3, u, :],
                        start=True, stop=True, tile_position=(0, 0),
                    )
                    nc.tensor.matmul(
                        pss, W0pack[64:72, half, :], rhs0[64:72, u, :],
                        start=True, stop=True, tile_position=(64, 0),
                    )
                    relu_epilogue(
                        hcur["d"][:, half, u, :], ps0,
                        Bias["d", 0][:, half : half + 1], ("d", 0, half, u),
                    )
                    relu_epilogue(
                        hcur["s"][:, half, u, :], pss,
                        Bias["s", 0][:, half : half + 1], ("s", 0, half, u),
                    )

            # ---- layers 1, 2: same weights serve both tiles back-to-back,
            # per-u psums + epilogues ----
            for li in (1, 2):
                hnext = {pfx: new_h(pfx, li + 1) for pfx in ("d", "s")}
                for half in range(2):
                    for pfx in ("d", "s"):
                        psu = [
                            ps_mm[pfx].tile([128, 512], F32, tag="mm", name="ps")
                            for _ in range(2)
                        ]
                        for c in range(2):
                            for u in range(2):
                                nc.tensor.matmul(
                                    psu[u],
                                    Wmid[pfx, li][:, c, half * 128 : half * 128 + 128],
                                    hcur[pfx][:, c, u, :],
                                    start=(c == 0),
                                    stop=(c == 1),
                                )
                        for u in range(2):
                            relu_epilogue(
                                hnext[pfx][:, half, u, :],
                                psu[u],
                                Bias[pfx, li][:, half : half + 1],
                                (pfx, li, half, u),
                            )
                hcur = hnext

            # ---- layer 3 (d at PE columns 0-3, s at columns 32-35, both
            # into one shared psum tile) + merged mask epilogues ----
            ps3 = ps_l3.tile([36, 2, 512], F32, tag="l3", name="ps3")
            for u in range(2):
                for c in range(2):
                    nc.tensor.matmul(
                        ps3[0:4, u, :],
                        W3["d"][:, c, :],
                        hcur["d"][:, c, u, :],
                        start=(c == 0), stop=(c == 1), tile_position=(0, 0),
                    )
                for c in range(2):
                    nc.tensor.matmul(
                        ps3[32:36, u, :],
                        W3["s"][:, c, :],
                        hcur["s"][:, c, u, :],
                        start=(c == 0), stop=(c == 1), tile_position=(0, 32),
                    )
            # d: ACT adds bias psum->sbuf, GpSimd applies the mask;
            # s: DVE does (psum + b) * mask in one op
            ot = pool_out.tile([3, 2, 512], F32, tag="otmp")
            nc.scalar.activation(
                ot,
                ps3[0:3, :, :],
                mybir.ActivationFunctionType.Identity,
                bias=Bias["d", 3][:, 0:1],
            )
            osb_d = pool_out.tile([3, 2, 512], F32, tag="od")
            nc.gpsimd.tensor_tensor(osb_d, ot, mb2, mybir.AluOpType.mult)
            osb_s = pool_out.tile([3, 2, 512], F32, tag="os")
            nc.vector.scalar_tensor_tensor(
                osb_s,
                ps3[32:35, :, :],
                Bias["s", 3][:, 0:1],
                mb2,
                mybir.AluOpType.add,
                mybir.AluOpType.mult,
            )
            for pfx, osb, outbuf in (("d", osb_d, out_d), ("s", osb_s, out_s)):
                nc.sync.dma_start(
                    outbuf[:, tp * 2 * TILE : (tp + 1) * 2 * TILE].rearrange(
                        "p (a b) -> p a b", b=TILE
                    ),
                    osb,
                )

    nc.compile()
    return nc


def _pack_weights(inputs):
    """Pad + fp16-cast the weight matrices once (shared across cores)."""
    w = {}
    d0 = np.asarray(inputs["dW0"], np.float32)  # [3, H]
    s0 = np.asarray(inputs["sW0"], np.float32)  # [8, H]
    pack = np.zeros((128, 2, 128), np.float32)
    for h in range(2):
        pack[0:3, h, :] = d0[:, h * 128 : h * 128 + 128]
        pack[64:72, h, :] = s0[:, h * 128 : h * 128 + 128]
    w["W0pack"] = pack.astype(np.float16)

    bf = np.float16
    for pfx in ("d", "s"):
        for li in (1, 2):
            w[f"{pfx}W{li}p"] = np.asarray(inputs[f"{pfx}W{li}"], dtype=bf)
        w[f"{pfx}W3p"] = np.asarray(
            np.concatenate(
                [inputs[f"{pfx}W3"], np.zeros((H, 1), np.float32)], axis=1
            ),
            dtype=bf,
        )  # [H, 4]
        for li in range(4):
            w[f"{pfx}b{li}"] = np.ascontiguousarray(
                inputs[f"{pfx}b{li}"], dtype=np.float32
            )
    return w


def get_nc(nt=DEFAULT_NT):
    key = ("nc", nt)
    if key not in _CACHE:
        _CACHE[key] = _build(nt)
    return _CACHE[key]


def _required_nt(nv):
    """Tiles per core needed for nv compacted points (rounded up to even)."""
    nt = -(-nv // (NCORES * TILE))
    nt += nt % 2
    return max(nt, 2)


def make_shards(inputs, nt=DEFAULT_NT):
    """Compact visible points, pad to nt tiles/core, build per-core shards.

    vis_idx is stashed in _CACHE for gather_outputs so the test harness's
    shard->run->gather flow works.
    """
    wpack = _pack_weights(inputs)
    pts_all = np.ascontiguousarray(
        np.concatenate(
            [
                np.asarray(inputs["normals"], np.float32),
                np.asarray(inputs["view_dirs"], np.float32),
                np.asarray(inputs["roughness"], np.float32),
                np.asarray(inputs["r0"], np.float32),
            ],
            axis=1,
        )
    )
    dot = np.einsum("ij,ij->i", pts_all[:, 0:3], pts_all[:, 3:6], dtype=np.float32)
    vis_idx = np.nonzero(dot > -DOT_MARGIN)[0]
    nv = len(vis_idx)
    need = _required_nt(nv)
    assert need <= nt, (
        f"visible points {nv} need {need} tiles/core but kernel built for {nt}"
    )
    ppc = nt * TILE
    pts_vis = np.zeros((NCORES * ppc, 8), np.float32)
    pts_vis[:nv] = pts_all[vis_idx]

    ident_bf = np.eye(128, dtype=np.float16)
    shards = []
    for i in range(NCORES):
        m = {"pts": pts_vis[i * ppc : (i + 1) * ppc], "identb": ident_bf}
        m.update(wpack)
        shards.append(m)
    _CACHE["vis_idx"] = vis_idx
    _CACHE["ppc"] = ppc
    return shards


def gather_outputs(results):
    vis_idx = _CACHE["vis_idx"]
    ppc = _CACHE["ppc"]
    nv = len(vis_idx)
    diff = np.zeros((P_FULL, 3), np.float32)
    spec = np.zeros((P_FULL, 3), np.float32)
    for i in range(NCORES):
        lo = i * ppc
        hi = min(lo + ppc, nv)
        if hi <= lo:
            break
        sl = vis_idx[lo:hi]
        diff[sl] = results[i]["out_d"][:, : hi - lo].T
        spec[sl] = results[i]["out_s"][:, : hi - lo].T
    return diff, spec


def kernel(**inputs):
    dot = np.einsum(
        "ij,ij->i",
        np.asarray(inputs["normals"], np.float32),
        np.asarray(inputs["view_dirs"], np.float32),
    )
    nv = int((dot > -DOT_MARGIN).sum())
    nt = max(_required_nt(nv), DEFAULT_NT)
    nc = get_nc(nt)
    shards = make_shards(inputs, nt)
    res = run_bass_kernel_spmd(nc, shards, core_ids=list(range(NCORES)))
    return gather_outputs(res.results)
